# revision 10
# baseline (speedup 1.0000x reference)
"""Trainium2 Bass kernel for MeshMultiHeadHodgeAttentionVertices (v2).

Strategy (8 cores, SPMD single NEFF, per-core data differs only in inputs):
  - Edge rows m sharded 8 ways (MS=1536/core); vertex rows n sharded 8 ways
    (NS=512/core).
  - One early AllGather of the combined Q/K table ([e_K b0|b1] for local
    edges + [v_Q b0|b1] for local vertices, 2048 rows x 1024B per core),
    launched right after the projections so it hides under phase 2.
  - Phase 3 is split: a score pass (gather K rows from the Q/K table,
    compute softmax weights -> awpe resident) and a value pass (gather X
    rows from the x_edges table after its own AllGather).
  - All neighbor gathers use SWDGE prepare_only + trigger_dma so the Q7
    engine only pays descriptor generation (~2us) per gather; transfers
    queue on the SDMA rings and overlap compute and each other.
  - AllReduce of x_vert is split into two row-halves launched mid-phase-4.
  - Heavy tensors cast to bf16 on host; accumulation in fp32 (PSUM);
    attention reduces use bf16 add-trees with fp32 final steps.
"""

import math
import os

import numpy as np
import ml_dtypes

import concourse.bass as bass
import concourse.bacc as bacc
import concourse.mybir as mybir
import concourse.tile as tile
from concourse.alu_op_type import AluOpType
from concourse.bass_utils import run_bass_kernel_spmd

BF16 = ml_dtypes.bfloat16
F32 = mybir.dt.float32
BF = mybir.dt.bfloat16
I16 = mybir.dt.int16

B, N, M, H, DV = 2, 4096, 12288, 8, 256
DK = DV // H            # 32
KNB = 16                # neighbors
W = 8                   # cores
MS = M // W             # 1536 edge rows per core
NS = N // W             # 512 vertex rows per core
MT = MS // 128          # 12 edge tiles
VT = NS // 128          # 4 vertex tiles
NT = N // 128           # 32 vertex tiles (full)
QKR = MS + NS           # 2048 rows per core in the combined Q/K table
LN_EPS = 1e-5
ISQ = 1.0 / math.sqrt(DK)

Exp = mybir.ActivationFunctionType.Exp
Sqrt = mybir.ActivationFunctionType.Sqrt
X = mybir.AxisListType.X

_CACHE: dict = {}


def _build_module(stage=5):
    nc = _emit(stage)
    nc.compile()
    return nc


def _emit(stage):
    nc = bacc.Bacc("TRN2", target_bir_lowering=False, debug=False,
                   num_devices=W)

    # ---- external inputs (per-core shards prepared on host) ----
    xvt_f = nc.dram_tensor("xvt_f", [B, DV, N], BF, kind="ExternalInput")
    xvt_s = nc.dram_tensor("xvt_s", [B, DV, NS], BF, kind="ExternalInput")
    xet_s = nc.dram_tensor("xet_s", [B, DV, MS], BF, kind="ExternalInput")
    w5 = nc.dram_tensor("w5", [5, DV, DV], BF, kind="ExternalInput")
    d0t = nc.dram_tensor("d0t", [B, N, MS], BF, kind="ExternalInput")
    d0n = nc.dram_tensor("d0n", [B, MS, N], BF, kind="ExternalInput")
    gieq = nc.dram_tensor("gieq", [128, MT * 128], I16, kind="ExternalInput")
    giex = nc.dram_tensor("giex", [128, MT * 128], I16, kind="ExternalInput")
    givq = nc.dram_tensor("givq", [128, VT * 128], I16, kind="ExternalInput")
    givx = nc.dram_tensor("givx", [128, VT * 128], I16, kind="ExternalInput")
    out = nc.dram_tensor("out", [B, NS, DV], F32, kind="ExternalOutput")

    rg = [list(range(W))]

    with tile.TileContext(nc) as tc:
        with (
            tc.tile_pool(name="dram", bufs=1, space="DRAM") as dram,
            tc.tile_pool(name="const", bufs=1) as constp,
            tc.tile_pool(name="resid", bufs=1) as resid,
        ):
            # collective buffers
            agqk_in = dram.tile([QKR, 2 * DV], BF, tag="agqki")
            agx_in = dram.tile([MS, 2 * DV], BF, tag="agxi")
            ar_in = [dram.tile([N, DV], BF, tag=f"ari{b}", name=f"ar_in{b}")
                     for b in range(B)]
            agqk_out = dram.tile([W * QKR, 2 * DV], BF, tag="agqko",
                                 addr_space="Shared")
            agx_out = dram.tile([M, 2 * DV], BF, tag="agxo",
                                addr_space="Shared")
            ar_out = [dram.tile([N, DV], BF, tag=f"aro{b}",
                                name=f"ar_out{b}", addr_space="Shared")
                      for b in range(B)]

            # constants / index tables
            wts = constp.tile([128, 5, 2, DV], BF, tag="wts")
            nc.sync.dma_start(
                out=wts[:], in_=w5[:].rearrange("w (hf p) e -> p w hf e", p=128))
            gieq_sb = constp.tile([128, MT, 128], I16, tag="gieq")
            nc.sync.dma_start(out=gieq_sb[:], in_=gieq[:].rearrange(
                "p (t s) -> p t s", t=MT))
            giex_sb = constp.tile([128, MT, 128], I16, tag="giex")
            nc.sync.dma_start(out=giex_sb[:], in_=giex[:].rearrange(
                "p (t s) -> p t s", t=MT))
            givq_sb = constp.tile([128, VT, 128], I16, tag="givq")
            nc.sync.dma_start(out=givq_sb[:], in_=givq[:].rearrange(
                "p (t s) -> p t s", t=VT))
            givx_sb = constp.tile([128, VT, 128], I16, tag="givx")
            nc.sync.dma_start(out=givx_sb[:], in_=givx[:].rearrange(
                "p (t s) -> p t s", t=VT))

            eps_t = constp.tile([128, 1], F32, tag="eps")
            nc.vector.memset(eps_t[:], LN_EPS)

            # residents that span multiple phases
            vk = resid.tile([128, B, VT, DV], BF, tag="vk")      # v_K shard
            eq = resid.tile([128, B, MT, DV], BF, tag="eq")      # e_Q shard
            ae = resid.tile([128, MT, B, DV], BF, tag="ae")      # attn edges
            awpe = resid.tile([128, MT, 2, KNB, H, 2], BF, tag="awpe")
            awpv = resid.tile([128, VT, 2, KNB, H, 2], BF, tag="awpv")

            sem_g = nc.alloc_semaphore("swdge_gather")

            def layernorm(src256, dst256, pool):
                """src256: (128,256) fp32 AP (psum); dst256: (128,256) bf16 AP.
                LN over groups of 32 along free dim."""
                s3 = src256.rearrange("p (h d) -> p h d", d=DK)
                sums = pool.tile([128, H], F32, tag="ln_sums")
                nc.vector.reduce_sum(sums[:], s3, axis=X)
                mean = pool.tile([128, H], F32, tag="ln_mean")
                nc.scalar.mul(mean[:], sums[:], 1.0 / DK)
                cent = pool.tile([128, H, DK], F32, tag="ln_cent")
                nc.vector.tensor_sub(
                    cent[:], s3,
                    mean[:].unsqueeze(2).broadcast_to((128, H, DK)))
                sq = pool.tile([128, H, DK], F32, tag="ln_sq")
                nc.scalar.square(sq[:], cent[:])
                vsum = pool.tile([128, H], F32, tag="ln_vsum")
                nc.vector.reduce_sum(vsum[:], sq[:], axis=X)
                std = pool.tile([128, H], F32, tag="ln_std")
                nc.scalar.activation(std[:], vsum[:], Sqrt,
                                     bias=eps_t[:], scale=1.0 / DK)
                rstd = pool.tile([128, H], F32, tag="ln_rstd")
                nc.vector.reciprocal(rstd[:], std[:])
                nc.vector.tensor_mul(
                    dst256.rearrange("p (h d) -> p h d", d=DK), cent[:],
                    rstd[:].unsqueeze(2).broadcast_to((128, H, DK)))

            use_prep = os.environ.get("KPREP", "0") == "1"

            def gather_prep(dst_ap, table_ap, idxs_ap, width):
                """Prepare+trigger a 2048-idx gather of `width`-elem bf16 rows
                into dst_ap (128, KNB, width). Q7 only pays desc-gen; the
                transfer runs on the SDMA rings, tracked by Tile via the
                prep's DMASW tick."""
                for h in range(2):
                    if use_prep:
                        nc.gpsimd.dma_gather(
                            out_ap=dst_ap[:, h * 8:(h + 1) * 8, :],
                            in_ap=table_ap,
                            idxs_ap=idxs_ap[:, h * 64:(h + 1) * 64],
                            num_idxs=1024, num_idxs_reg=1024,
                            elem_size=width,
                            prepare_only=True, sem=sem_g)
                        nc.gpsimd.trigger_dma(count=None)
                    else:
                        nc.gpsimd.dma_gather(
                            out_ap=dst_ap[:, h * 8:(h + 1) * 8, :],
                            in_ap=table_ap,
                            idxs_ap=idxs_ap[:, h * 64:(h + 1) * 64],
                            num_idxs=1024, num_idxs_reg=1024,
                            elem_size=width)

            def attn_scores_a(pool, kk, qq):
                """Both batches at once: scores + exp.  kk: (128, KNB, 2,
                H, DK) bf16 AP (gathered K rows); qq: (128, 2, H, DK)
                bf16 AP.  Returns the pexp tile (exp issued on ACT)."""
                q5 = qq.unsqueeze(1).broadcast_to((128, KNB, 2, H, DK))
                tmp = pool.tile([128, KNB, 2, H, DK], BF, tag="sc_tmp",
                                bufs=1)
                nc.vector.tensor_mul(tmp[:], kk, q5)
                # reduce over d=32 via add tree (bf16, final step fp32)
                t1 = pool.tile([128, KNB, 2, H, 16], BF, tag="sc_t1", bufs=1)
                nc.vector.tensor_add(t1[:], tmp[:, :, :, :, 0:16],
                                     tmp[:, :, :, :, 16:32])
                t2 = pool.tile([128, KNB, 2, H, 8], BF, tag="sc_t2", bufs=1)
                nc.vector.tensor_add(t2[:], t1[:, :, :, :, 0:8],
                                     t1[:, :, :, :, 8:16])
                t3 = pool.tile([128, KNB, 2, H, 4], BF, tag="sc_t3", bufs=1)
                nc.vector.tensor_add(t3[:], t2[:, :, :, :, 0:4],
                                     t2[:, :, :, :, 4:8])
                t4 = pool.tile([128, KNB, 2, H, 2], BF, tag="sc_t4", bufs=1)
                nc.vector.tensor_add(t4[:], t3[:, :, :, :, 0:2],
                                     t3[:, :, :, :, 2:4])
                scores = pool.tile([128, KNB, 2, H], F32, tag="sc_sc")
                nc.vector.tensor_add(scores[:], t4[:, :, :, :, 0],
                                     t4[:, :, :, :, 1])
                pexp = pool.tile([128, KNB, 2, H], F32, tag="sc_pe")
                nc.scalar.activation(pexp[:], scores[:], Exp, scale=ISQ)
                return pexp

            def attn_scores_b(pool, pexp, awp):
                """Softmax normalization: pexp (128, KNB, 2, H) f32 tile ->
                awp (128, 2, KNB, H, 2) duplicated weights."""
                rsum = pool.tile([128, 2, H], F32, tag="sc_rs")
                nc.vector.reduce_sum(
                    rsum[:], pexp[:].rearrange("p j b h -> p b h j"), axis=X)
                rrec = pool.tile([128, 2, H], F32, tag="sc_rr")
                nc.vector.reciprocal(rrec[:], rsum[:])
                for b in range(2):
                    rrb = rrec[:, b].unsqueeze(1) \
                        .broadcast_to((128, KNB, H))
                    nc.vector.tensor_mul(awp[:, b, :, :, 0],
                                         pexp[:, :, b, :], rrb)
                    nc.vector.tensor_mul(awp[:, b, :, :, 1],
                                         pexp[:, :, b, :], rrb)

            def attn_values(pool, xx0, xx1, awp, res2):
                """xx0/xx1: (128, KNB, 256) bf16 value rows per batch; awp:
                (128, 2, KNB, H, 2); res2: (128, 2, H, DK) fp32 AP."""
                tmp2 = pool.tile([128, KNB, 2, H, DK], BF, tag="va_tmp",
                                 bufs=1)
                for b, xx in ((0, xx0), (1, xx1)):
                    x5 = xx.rearrange("p j (h x y) -> p j h x y", y=2, x=16)
                    o5 = tmp2[:, :, b].rearrange(
                        "p j h (x y) -> p j h x y", y=2)
                    aw5 = awp[:, b].unsqueeze(3) \
                        .broadcast_to((128, KNB, H, DK // 2, 2))
                    nc.vector.tensor_mul(o5, x5, aw5)
                # reduce over j=16 via add tree
                v1 = pool.tile([128, 8, 2, H, DK], BF, tag="va_v1", bufs=1)
                nc.vector.tensor_add(v1[:], tmp2[:, 0:8], tmp2[:, 8:16])
                v2 = pool.tile([128, 4, 2, H, DK], BF, tag="va_v2", bufs=1)
                nc.vector.tensor_add(v2[:], v1[:, 0:4], v1[:, 4:8])
                v3 = pool.tile([128, 2, 2, H, DK], BF, tag="va_v3", bufs=1)
                nc.vector.tensor_add(v3[:], v2[:, 0:2], v2[:, 2:4])
                nc.vector.tensor_add(res2, v3[:, 0], v3[:, 1])

            # ---------------- phase 1: projections --------------------
            with tc.tile_pool(name="vvp", bufs=1) as vvp:
                vv = vvp.tile([128, B, NT, DV], BF, tag="vv")    # v_V full
                with (
                    tc.tile_pool(name="acts", bufs=1) as actsp,
                    tc.tile_pool(name="proj", bufs=2) as projp,
                    tc.tile_pool(name="lnp", bufs=3) as lnp,
                    tc.tile_pool(name="ps1", bufs=2, space="PSUM") as ps1,
                    tc.tile_pool(name="ps2", bufs=2, space="PSUM") as ps2,
                ):
                    xvf_sb = actsp.tile([128, B, 2, N], BF, tag="xvf")
                    nc.sync.dma_start(
                        out=xvf_sb[:],
                        in_=xvt_f[:].rearrange("b (hf p) n -> p b hf n", p=128))
                    xvs_sb = actsp.tile([128, B, 2, NS], BF, tag="xvs")
                    nc.sync.dma_start(
                        out=xvs_sb[:],
                        in_=xvt_s[:].rearrange("b (hf p) n -> p b hf n", p=128))
                    xes_sb = actsp.tile([128, B, 2, MS], BF, tag="xes")
                    nc.sync.dma_start(
                        out=xes_sb[:],
                        in_=xet_s[:].rearrange("b (hf p) n -> p b hf n", p=128))
                    # v_Q / v_K; v_Q rows go into the combined Q/K table
                    for b in range(B):
                        for vt in range(VT):
                            psqk = ps2.tile([128, 2 * DV], F32, tag="psqk")
                            for hf in range(2):
                                nc.tensor.matmul(
                                    psqk[:],
                                    lhsT=xvs_sb[:, b, hf,
                                                vt * 128:(vt + 1) * 128],
                                    rhs=wts[:, 0:2, hf, :],
                                    start=(hf == 0), stop=(hf == 1))
                            vq_t = projp.tile([128, DV], BF, tag="vq_t")
                            layernorm(psqk[:, 0:DV], vq_t[:], lnp)
                            r0 = MS + vt * 128
                            nc.sync.dma_start(
                                out=agqk_in[r0:r0 + 128, b * DV:(b + 1) * DV],
                                in_=vq_t[:])
                            layernorm(psqk[:, DV:2 * DV], vk[:, b, vt, :], lnp)
                    # e_Q / e_K for edge shard tiles
                    for b in range(B):
                        for mt in range(MT):
                            psek = ps2.tile([128, 2 * DV], F32, tag="psqk")
                            for hf in range(2):
                                nc.tensor.matmul(
                                    psek[:],
                                    lhsT=xes_sb[:, b, hf,
                                                mt * 128:(mt + 1) * 128],
                                    rhs=wts[:, 3:5, hf, :],
                                    start=(hf == 0), stop=(hf == 1))
                            layernorm(psek[:, 0:DV], eq[:, b, mt, :], lnp)
                            ek_t = projp.tile([128, DV], BF, tag="ek_t")
                            layernorm(psek[:, DV:2 * DV], ek_t[:], lnp)
                            nc.sync.dma_start(
                                out=agqk_in[mt * 128:(mt + 1) * 128,
                                            b * DV:(b + 1) * DV],
                                in_=ek_t[:])
                    # combined Q/K AllGather (hides under v_V + phase 2)
                    nc.gpsimd.collective_compute(
                        "AllGather", AluOpType.bypass, replica_groups=rg,
                        ins=[agqk_in[:].opt()], outs=[agqk_out[:].opt()])
                    # v_V for all vertex tiles
                    for b in range(B):
                        for nt in range(NT):
                            psv = ps1.tile([128, DV], F32, tag="psv")
                            for hf in range(2):
                                nc.tensor.matmul(
                                    psv[:],
                                    lhsT=xvf_sb[:, b, hf,
                                                nt * 128:(nt + 1) * 128],
                                    rhs=wts[:, 2, hf, :],
                                    start=(hf == 0), stop=(hf == 1))
                            nc.scalar.copy(vv[:, b, nt, :], psv[:])

                if stage < 1:
                    with tc.tile_pool(name="dbgp", bufs=2) as dbgp:
                        for b in range(B):
                            for vt in range(VT):
                                dbg = dbgp.tile([128, DV], F32, tag="dbg")
                                nc.scalar.copy(dbg[:], vk[:, b, vt, :])
                                nc.sync.dma_start(
                                    out=out[b, vt * 128:(vt + 1) * 128, :],
                                    in_=dbg[:])
                    return nc

                # ------------- phase 2: x_edges = d_0 @ v_V -------------
                with (
                    tc.tile_pool(name="s4p", bufs=3) as s4p,
                    tc.tile_pool(name="ps4", bufs=2, space="PSUM") as ps4p,
                ):
                    for mtp in range(MT // 2):
                        for b in range(B):
                            dt_t = s4p.tile([128, NT, 256], BF, tag="d0t",
                                            bufs=2)
                            nc.sync.dma_start(
                                out=dt_t[:],
                                in_=d0t[b].rearrange(
                                    "(nt p) m -> p nt m", p=128)
                                [:, :, mtp * 256:(mtp + 1) * 256])
                            for sub in range(2):
                                mt = mtp * 2 + sub
                                ps4 = ps4p.tile([128, DV], F32, tag="ps4")
                                for nt in range(NT):
                                    nc.tensor.matmul(
                                        ps4[:],
                                        lhsT=dt_t[:, nt,
                                                  sub * 128:(sub + 1) * 128],
                                        rhs=vv[:, b, nt, :],
                                        start=(nt == 0), stop=(nt == NT - 1))
                                xe_t = s4p.tile([128, DV], BF, tag="xe_t")
                                nc.scalar.copy(xe_t[:], ps4[:])
                                nc.sync.dma_start(
                                    out=agx_in[mt * 128:(mt + 1) * 128,
                                               b * DV:(b + 1) * DV],
                                    in_=xe_t[:])
                # x_edges AllGather
                nc.gpsimd.collective_compute(
                    "AllGather", AluOpType.bypass, replica_groups=rg,
                    ins=[agx_in[:].opt()], outs=[agx_out[:].opt()])

            if stage < 2:
                with tc.tile_pool(name="dbg2p", bufs=2) as dbg2p:
                    for b in range(B):
                        for vt in range(VT):
                            dbg2 = dbg2p.tile([128, DV], BF, tag="dbg2")
                            nc.sync.dma_start(
                                out=dbg2[:],
                                in_=agx_out[vt * 128:(vt + 1) * 128,
                                            b * DV:(b + 1) * DV])
                            dbg2f = dbg2p.tile([128, DV], F32, tag="dbg2f")
                            nc.scalar.copy(dbg2f[:], dbg2[:])
                            nc.sync.dma_start(
                                out=out[b, vt * 128:(vt + 1) * 128, :],
                                in_=dbg2f[:])
                return nc

            # ------------- phase 3a: edge attention scores -------------
            with (
                tc.tile_pool(name="kgp", bufs=3) as kgp,
                tc.tile_pool(name="scp", bufs=2) as scp,
            ):
                for mt in range(MT):
                    kg = kgp.tile([128, KNB, 2 * DV], BF, tag="kg")
                    gather_prep(kg[:], agqk_out[:], gieq_sb[:, mt, :], 2 * DV)
                    pexp = attn_scores_a(
                        scp,
                        kg[:].rearrange(
                            "p j (b h d) -> p j b h d", b=2, d=DK),
                        eq[:, :, mt, :].rearrange(
                            "p b (h d) -> p b h d", d=DK))
                    attn_scores_b(scp, pexp, awpe[:, mt])

            if stage < 3:
                with tc.tile_pool(name="dbg3p", bufs=2) as dbg3p:
                    for b in range(B):
                        for vt in range(VT):
                            dbg3 = dbg3p.tile([128, KNB, H, 2], F32,
                                              tag="dbg3")
                            nc.scalar.copy(dbg3[:], awpe[:, vt, b])
                            nc.sync.dma_start(
                                out=out[b, vt * 128:(vt + 1) * 128, :],
                                in_=dbg3[:].rearrange("p j h y -> p (j h y)"))
                return nc

            # ------------- phase 3b: edge attention values -------------
            with (
                tc.tile_pool(name="xgp", bufs=3) as xgp,
                tc.tile_pool(name="vap", bufs=2) as vap,
            ):
                for mt in range(MT):
                    xg = xgp.tile([128, KNB, 2 * DV], BF, tag="xg")
                    gather_prep(xg[:], agx_out[:], giex_sb[:, mt, :], 2 * DV)
                    res2 = vap.tile([128, 2, H, DK], F32, tag="eres")
                    attn_values(vap,
                                xg[:, :, 0:DV],
                                xg[:, :, DV:2 * DV],
                                awpe[:, mt], res2[:])
                    nc.scalar.copy(
                        ae[:, mt, :, :],
                        res2[:].rearrange("p b h d -> p b (h d)"))

            # ------------- vertex attention scores ---------------------
            with (
                tc.tile_pool(name="vqg", bufs=2) as vqgp,
                tc.tile_pool(name="vsp", bufs=2) as vsp,
            ):
                for vt in range(VT):
                    vqt = vqgp.tile([128, KNB, 2 * DV], BF, tag="vqt")
                    gather_prep(vqt[:], agqk_out[:], givq_sb[:, vt, :],
                                2 * DV)
                    pexp_v = attn_scores_a(
                        vsp,
                        vqt[:].rearrange(
                            "p j (b h d) -> p j b h d", b=2, d=DK),
                        vk[:, :, vt, :].rearrange(
                            "p b (h d) -> p b h d", d=DK))
                    attn_scores_b(vsp, pexp_v, awpv[:, vt])

            # ------------- phase 4: x_vert partial = d0n^T @ ae --------
            with (
                tc.tile_pool(name="s6p", bufs=4) as s6p,
                tc.tile_pool(name="ps6", bufs=2, space="PSUM") as ps6p,
            ):
                for b in range(B):
                    for ck in range(NT // 8):
                        dts = []
                        for mt in range(MT):
                            dn_t = s6p.tile([128, 8 * 128], BF,
                                            tag="d0n", bufs=16)
                            nc.sync.dma_start(
                                out=dn_t[:],
                                in_=d0n[b, mt * 128:(mt + 1) * 128,
                                        ck * 1024:(ck + 1) * 1024])
                            dts.append(dn_t)
                        # two 4-bank accumulation groups (one group per
                        # PSUM generation; start= must own its banks)
                        for gr in range(2):
                            pss = [ps6p.tile([128, DV], F32,
                                             tag=f"s6_{i}",
                                             name=f"s6_{ck}_{b}_{gr}_{i}")
                                   for i in range(4)]
                            for mt in range(MT):
                                for i4 in range(4):
                                    i = gr * 4 + i4
                                    nc.tensor.matmul(
                                        pss[i4][:],
                                        lhsT=dts[mt][:,
                                                     i * 128:(i + 1) * 128],
                                        rhs=ae[:, mt, b, :],
                                        start=(mt == 0),
                                        stop=(mt == MT - 1))
                            for i4 in range(4):
                                i = gr * 4 + i4
                                xv_t = s6p.tile([128, DV], BF, tag="xv_t")
                                nc.scalar.copy(xv_t[:], pss[i4][:])
                                r0 = (ck * 8 + i) * 128
                                nc.sync.dma_start(
                                    out=ar_in[b][r0:r0 + 128, :],
                                    in_=xv_t[:])
                    # AllReduce this batch's partials as soon as stored;
                    # b=0's AllReduce overlaps b=1's matmuls
                    nc.gpsimd.collective_compute(
                        "AllReduce", AluOpType.add, replica_groups=rg,
                        ins=[ar_in[b][:].opt()],
                        outs=[ar_out[b][:].opt()])

            if stage < 5:
                with tc.tile_pool(name="dbg4p", bufs=2) as dbg4p:
                    for vt in range(VT):
                        for b in range(B):
                            g = dbg4p.tile([128, KNB, DV], BF, tag="dbg4")
                            gather_prep(g[:], ar_out[b][:],
                                        givx_sb[:, vt, :], DV)
                            dbg4f = dbg4p.tile([128, DV], F32, tag="dbg4f")
                            nc.scalar.copy(dbg4f[:], g[:, 0, :])
                            nc.sync.dma_start(
                                out=out[b, vt * 128:(vt + 1) * 128, :],
                                in_=dbg4f[:])
                return nc

            # ------------- phase 5: vertex attention (values only) -----
            with (
                tc.tile_pool(name="vat", bufs=2) as vat,
                tc.tile_pool(name="vatw", bufs=2) as vatw,
            ):
                for vt in range(VT):
                    xg0 = vat.tile([128, KNB, DV], BF, tag="xg_v0")
                    gather_prep(xg0[:], ar_out[0][:], givx_sb[:, vt, :], DV)
                    xg1 = vat.tile([128, KNB, DV], BF, tag="xg_v1")
                    gather_prep(xg1[:], ar_out[1][:], givx_sb[:, vt, :], DV)
                    res2 = vatw.tile([128, 2, H, DK], F32, tag="vres")
                    attn_values(vatw, xg0[:], xg1[:],
                                awpv[:, vt], res2[:])
                    for b in range(B):
                        nc.sync.dma_start(
                            out=out[b, vt * 128:(vt + 1) * 128, :],
                            in_=res2[:, b].rearrange("p h d -> p (h d)"))

    return nc


def _pack_idx(L):
    """L: (T, n) int array of table-row indices (j-major per tile) ->
    (128, T*(n//16)) int16 dma_gather index layout (16-part wrap, 8x repl)."""
    T, n = L.shape
    a = L.reshape(T, n // 16, 16).transpose(2, 0, 1).reshape(16, T * (n // 16))
    return np.tile(a, (8, 1)).astype(np.int16)


def _prep_core_inputs(c, x_v, x_e, d_0, w5_bf, v_idx, e_idx, xvt_full):
    sh_e = slice(c * MS, (c + 1) * MS)
    sh_v = slice(c * NS, (c + 1) * NS)

    d0s = d_0[:, sh_e, :]
    d0n_c = np.ascontiguousarray(d0s).astype(BF16)
    d0t_c = np.ascontiguousarray(d0s.transpose(0, 2, 1)).astype(BF16)
    xvt_s = np.ascontiguousarray(
        x_v[:, sh_v, :].transpose(0, 2, 1)).astype(BF16)
    xet_s = np.ascontiguousarray(
        x_e[:, sh_e, :].transpose(0, 2, 1)).astype(BF16)

    # edge neighbor rows, j-major per 128-row tile
    e = e_idx[sh_e].astype(np.int64)
    Le = e.reshape(MT, 128, KNB).transpose(0, 2, 1).reshape(MT, KNB * 128)
    v = v_idx[sh_v].astype(np.int64).reshape(VT, 128, KNB)
    Lv = v.transpose(0, 2, 1).reshape(VT, KNB * 128)

    # plain tables: x_edges AG output is rank-major = global edge id;
    # ar output rows are global vertex ids
    giex_np = _pack_idx(Le)
    givx_np = _pack_idx(Lv)
    # combined Q/K table: rank r holds rows [r*QKR, r*QKR+MS) = edges,
    # [r*QKR+MS, (r+1)*QKR) = vertices
    Leq = (Le // MS) * QKR + (Le % MS)
    Lvq = (Lv // NS) * QKR + MS + (Lv % NS)
    gieq_np = _pack_idx(Leq)
    givq_np = _pack_idx(Lvq)

    return {
        "xvt_f": xvt_full,
        "xvt_s": xvt_s,
        "xet_s": xet_s,
        "w5": w5_bf,
        "d0t": d0t_c,
        "d0n": d0n_c,
        "gieq": gieq_np,
        "giex": giex_np,
        "givq": givq_np,
        "givx": givx_np,
    }


def run(inputs, trace=False):
    x_v = np.asarray(inputs["x_v"], np.float32)
    x_e = np.asarray(inputs["x_e"], np.float32)
    d_0 = np.asarray(inputs["d_0"], np.float32)
    v_idx = np.asarray(inputs["v_idx"])
    e_idx = np.asarray(inputs["e_idx"])
    w5_bf = np.ascontiguousarray(np.stack([
        np.asarray(inputs["W_vQ"]).T, np.asarray(inputs["W_vK"]).T,
        np.asarray(inputs["W_vV"]).T, np.asarray(inputs["W_eQ"]).T,
        np.asarray(inputs["W_eK"]).T])).astype(BF16)
    xvt_full = np.ascontiguousarray(x_v.transpose(0, 2, 1)).astype(BF16)

    stage = int(os.environ.get("KSTAGE", "5"))
    if ("nc", stage) not in _CACHE:
        _CACHE[("nc", stage)] = _build_module(stage)
    nc = _CACHE[("nc", stage)]

    in_maps = [
        _prep_core_inputs(c, x_v, x_e, d_0, w5_bf, v_idx, e_idx, xvt_full)
        for c in range(W)
    ]
    try:
        r = run_bass_kernel_spmd(nc, in_maps, core_ids=list(range(W)),
                                 trace=trace)
    except ModuleNotFoundError:
        r = run_bass_kernel_spmd(nc, in_maps, core_ids=list(range(W)),
                                 trace=False)
    outs = [r.results[c]["out"] for c in range(W)]
    full = np.concatenate(outs, axis=1).astype(np.float32)
    return full, r.exec_time_ns


def kernel(**inputs):
    full, _ = run(inputs, trace=False)
    return full
